# revision 22
# baseline (speedup 1.0000x reference)
"""Cross-attention fusion kernel for Trainium2 (8 NeuronCores).

Reference computation (per sample b):
    q = Wq @ xs + bq            xs = x_s2[b] as [256, 4096]
    k = Wk @ xd + bk            xd = x_dem[b] as [64, 4096]
    v = Wv @ xd + bv
    attn = softmax_j(k^T q * c)             c = 256 ** -0.5
    out = v @ attn + x_s2[b]                out[ch, j] = sum_i v[ch, i] attn[i, j]

Device-side restructure (mathematically identical):
  - kq = (Wq * c)^T @ k, so logits = kq^T @ xs and q never materializes.
  - bq adds a per-i constant to logits, which cancels in softmax_j -> dropped.
  - bk / bv folded in by augmenting xd with a ones row (contraction K=65).
  - softmax denominators folded into v columns (scale v[:, i] by 1/sum_j e[i, j])
    instead of scaling the whole e matrix.
  - exp logits are left unshifted (no running-max): logits are O(1) here and
    fp32 exp has ~1e38 of headroom.

Sharding: 8 cores = 4 samples x 2 halves of the key-pixel axis i. Each core
emits a partial out [256, 4096]; the host sums the two halves and adds the
residual. No collectives.
"""

import numpy as np
import ml_dtypes

import concourse.bass as bass
import concourse.mybir as mybir
import concourse.tile as tile
from concourse import bacc
from concourse.bass_utils import run_bass_kernel_spmd

P = 128
CH = 256          # out_ch == s2_ch
DEM = 64          # dem_ch
N = 4096          # pixels per sample (j axis)
NI = 2048         # key pixels per core (i axis, half of N)
KO = CH // P      # 2 partition chunks of the 256-channel axis
NIB = NI // P     # 16 i-blocks per core
NCORES = 8

F32 = mybir.dt.float32
BF16 = mybir.dt.bfloat16
NP_BF16 = ml_dtypes.bfloat16


def build_bass():
    nc = bacc.Bacc(None, target_bir_lowering=False)

    xs_d = nc.dram_tensor("xs", [CH, N], BF16, kind="ExternalInput")
    xda_d = nc.dram_tensor("xda", [DEM + 1, NI], BF16, kind="ExternalInput")
    wq_d = nc.dram_tensor("wq", [CH, CH], BF16, kind="ExternalInput")
    wk_d = nc.dram_tensor("wk", [DEM + 1, CH], BF16, kind="ExternalInput")
    wv_d = nc.dram_tensor("wv", [DEM + 1, CH], BF16, kind="ExternalInput")
    out_d = nc.dram_tensor("out", [CH, N], F32, kind="ExternalOutput")

    xs_v = xs_d.ap().rearrange("(ko p) j -> p ko j", p=P)
    wq_v = wq_d.ap().rearrange("(ko p) m -> p ko m", p=P)
    out_v = out_d.ap().rearrange("(m p) j -> p m j", p=P)

    with tile.TileContext(nc) as tc:
        with (
            tc.tile_pool(name="consts", bufs=1) as consts,
            tc.tile_pool(name="bigs", bufs=1) as bigs,
            tc.tile_pool(name="small", bufs=1) as small,
            tc.tile_pool(name="stage", bufs=4) as stage,
        ):
            # Phase A needs only wk + xda: issue those DMAs first.
            wk_sb = consts.tile([DEM + 1, CH], BF16)
            nc.sync.dma_start(out=wk_sb, in_=wk_d.ap())
            xda_sb = consts.tile([DEM + 1, NI], BF16)
            nc.sync.dma_start(out=xda_sb, in_=xda_d.ap())
            wv_sb = consts.tile([DEM + 1, CH], BF16)
            nc.sync.dma_start(out=wv_sb, in_=wv_d.ap())
            wq_sb = consts.tile([P, KO, CH], BF16)
            nc.sync.dma_start(out=wq_sb, in_=wq_v)

            xs_sb = bigs.tile([P, KO, N], BF16)
            for ko in range(KO):
                nc.sync.dma_start(out=xs_sb[:, ko, :], in_=xs_v[:, ko, :])

            k_sb = bigs.tile([P, KO, NI], BF16)    # k[o, i], o on partitions
            kq_sb = bigs.tile([P, KO, NI], BF16)   # kq[ci, i], ci on partitions
            vt_sb = bigs.tile([P, NIB, CH], BF16)  # v^T[i, ch], i on partitions
            e_sb = bigs.tile([P, NIB, N], BF16)    # exp(logits)[i, j]

            r_sb = small.tile([P, NIB], F32)

            with tc.tile_pool(name="mm_psum", bufs=2, space="PSUM") as mm_psum:
                # ---- Phase A: k = [Wk^T; bk]^T @ [xd; 1] -> k_sb [o, i] ----
                for m in range(KO):
                    ps = mm_psum.tile([P, 2048], F32, tag="ps")
                    for i4 in range(NI // 512):
                        nc.tensor.matmul(
                            ps[:, i4 * 512:(i4 + 1) * 512],
                            lhsT=wk_sb[:, m * P:(m + 1) * P],
                            rhs=xda_sb[:, i4 * 512:(i4 + 1) * 512],
                            start=True, stop=True,
                        )
                    nc.vector.tensor_copy(out=k_sb[:, m, :], in_=ps)

                # ---- Phase B: v^T = [xd; 1]^T @ [Wv^T; bv] -> vt_sb ----
                for i4 in range(4):
                    ps = mm_psum.tile([P, 2048], F32, tag="ps")
                    for q in range(4):
                        ib = i4 * 4 + q
                        nc.tensor.matmul(
                            ps[:, q * 512:q * 512 + CH],
                            lhsT=xda_sb[:, ib * P:(ib + 1) * P],
                            rhs=wv_sb,
                            start=True, stop=True,
                        )
                        nc.vector.tensor_copy(
                            out=vt_sb[:, ib, :], in_=ps[:, q * 512:q * 512 + CH]
                        )

                # ---- Phase C: kq[ci, i] = sum_o (Wq*c)[o, ci] k[o, i] ----
                for m in range(KO):
                    ps = mm_psum.tile([P, 2048], F32, tag="ps")
                    for ko in range(KO):
                        for jj in range(4):
                            s0 = jj * 512
                            nc.tensor.matmul(
                                ps[:, s0:s0 + 512],
                                lhsT=wq_sb[:, ko, m * P:(m + 1) * P],
                                rhs=k_sb[:, ko, s0:s0 + 512],
                                start=(ko == 0), stop=(ko == 1),
                            )
                    nc.vector.tensor_copy(out=kq_sb[:, m, :], in_=ps)

                # ---- Phase D: logits -> exp -> row sums -> scale v^T ----
                for ib in range(NIB):
                    for jp in range(N // 2048):
                        pp = mm_psum.tile([P, 2048], F32, tag="ps")
                        for ko in range(KO):
                            for jj in range(4):
                                j0 = jp * 2048 + jj * 512
                                nc.tensor.matmul(
                                    pp[:, jj * 512:(jj + 1) * 512],
                                    lhsT=kq_sb[:, ko, ib * P:(ib + 1) * P],
                                    rhs=xs_sb[:, ko, j0:j0 + 512],
                                    start=(ko == 0), stop=(ko == 1),
                                )
                        nc.scalar.activation(
                            out=e_sb[:, ib, jp * 2048:(jp + 1) * 2048],
                            in_=pp,
                            func=mybir.ActivationFunctionType.Exp,
                        )
                    # row sums via a 4x-mode identity tensor_scalar with
                    # accumulate output (much faster than 1x tensor_reduce)
                    nc.vector.tensor_scalar(
                        out=e_sb[:, ib, :],
                        in0=e_sb[:, ib, :],
                        scalar1=1.0,
                        scalar2=0.0,
                        op0=mybir.AluOpType.mult,
                        op1=mybir.AluOpType.add,
                        accum_out=r_sb[:, ib:ib + 1],
                    )
                    nc.vector.reciprocal(
                        out=r_sb[:, ib:ib + 1], in_=r_sb[:, ib:ib + 1]
                    )
                    nc.vector.tensor_scalar_mul(
                        out=vt_sb[:, ib, :],
                        in0=vt_sb[:, ib, :],
                        scalar1=r_sb[:, ib:ib + 1],
                    )

            # ---- Phase E: out[ch, j] = sum_i vts[i, ch] e[i, j] ----
            with tc.tile_pool(name="out_psum", bufs=4, space="PSUM") as out_psum:
                for jq in range(4):
                    pq = [out_psum.tile([P, 512], F32, tag="po",
                                        name=f"po_{jq}_{t}")
                          for t in range(4)]
                    for ib in range(NIB):
                        for m in range(KO):
                            for jj in range(2):
                                jn = jq * 2 + jj
                                nc.tensor.matmul(
                                    pq[m * 2 + jj],
                                    lhsT=vt_sb[:, ib, m * P:(m + 1) * P],
                                    rhs=e_sb[:, ib, jn * 512:(jn + 1) * 512],
                                    start=(ib == 0), stop=(ib == NIB - 1),
                                )
                    for m in range(KO):
                        for jj in range(2):
                            jn = jq * 2 + jj
                            st = stage.tile([P, 512], F32, tag="st")
                            nc.vector.tensor_copy(out=st, in_=pq[m * 2 + jj])
                            nc.sync.dma_start(
                                out=out_v[:, m, jn * 512:(jn + 1) * 512], in_=st
                            )
    nc.finalize()
    return nc


_NC_CACHE = None


def _get_nc():
    global _NC_CACHE
    if _NC_CACHE is None:
        _NC_CACHE = build_bass()
    return _NC_CACHE


def make_in_maps(x_s2, x_dem, Wq, bq, Wk, bk, Wv, bv):
    scale = np.float32(CH ** -0.5)
    wq = np.ascontiguousarray(Wq * scale).astype(NP_BF16)                # [o, ci]
    wk = np.concatenate([Wk.T, bk[None, :]], axis=0).astype(NP_BF16)     # [65, 256]
    wv = np.concatenate([Wv.T, bv[None, :]], axis=0).astype(NP_BF16)
    ones = np.ones((1, NI), np.float32)
    in_maps = []
    for c in range(NCORES):
        s, h = divmod(c, 2)
        xs = np.ascontiguousarray(x_s2[s].reshape(CH, N)).astype(NP_BF16)
        xd = x_dem[s].reshape(DEM, N)[:, h * NI:(h + 1) * NI]
        xda = np.concatenate([xd, ones], axis=0).astype(NP_BF16)
        in_maps.append({"xs": xs, "xda": np.ascontiguousarray(xda),
                        "wq": wq, "wk": wk, "wv": wv})
    return in_maps


def run(inputs, trace=False, trace_cores=None):
    """Run the device kernel; returns (output, BassKernelResults)."""
    x_s2 = np.asarray(inputs["x_s2"], np.float32)
    x_dem = np.asarray(inputs["x_dem"], np.float32)
    args = {k: np.asarray(inputs[k], np.float32)
            for k in ("Wq", "bq", "Wk", "bk", "Wv", "bv")}
    in_maps = make_in_maps(x_s2, x_dem, args["Wq"], args["bq"],
                           args["Wk"], args["bk"], args["Wv"], args["bv"])
    nc = _get_nc()
    res = run_bass_kernel_spmd(nc, in_maps, core_ids=list(range(NCORES)),
                               trace=trace, trace_cores=trace_cores)
    B = x_s2.shape[0]
    out = np.empty_like(x_s2)
    for s in range(B):
        part = res.results[2 * s]["out"] + res.results[2 * s + 1]["out"]
        out[s] = part.reshape(CH, 64, 64) + x_s2[s]
    return out, res


def kernel(**inputs):
    out, _ = run(inputs, trace=False)
    return out


# revision 28
# speedup vs baseline: 1.2263x; 1.2263x over previous
"""Cross-attention fusion kernel for Trainium2 (8 NeuronCores).

Reference computation (per sample b):
    q = Wq @ xs + bq            xs = x_s2[b] as [256, 4096]
    k = Wk @ xd + bk            xd = x_dem[b] as [64, 4096]
    v = Wv @ xd + bv
    attn = softmax_j(k^T q * c)             c = 256 ** -0.5
    out = v @ attn + x_s2[b]                out[ch, j] = sum_i v[ch, i] attn[i, j]

Device-side restructure (mathematically identical):
  - kq = (Wq * c)^T @ k, so logits = kq^T @ xs and q never materializes.
  - bq adds a per-i constant to logits, which cancels in softmax_j -> dropped.
  - bk / bv folded in by augmenting xd with a ones row (contraction K=65).
  - softmax denominators folded into v columns (scale v[:, i] by 1/sum_j e[i, j])
    instead of scaling the whole e matrix.
  - exp logits are left unshifted (no running-max): logits are O(1) here and
    fp32 exp has ~1e38 of headroom.

Sharding: 8 cores = 4 samples x 2 halves of the key-pixel axis i. Each core
emits a partial out [256, 4096]; the host sums the two halves and adds the
residual. No collectives.
"""

import numpy as np
import ml_dtypes

import concourse.bass as bass
import concourse.mybir as mybir
import concourse.tile as tile
from concourse import bacc
from concourse.bass_utils import run_bass_kernel_spmd

P = 128
CH = 256          # out_ch == s2_ch
DEM = 64          # dem_ch
N = 4096          # pixels per sample (j axis)
NI = 2048         # key pixels per core (i axis, half of N)
KO = CH // P      # 2 partition chunks of the 256-channel axis
NIB = NI // P     # 16 i-blocks per core
NCORES = 8

F32 = mybir.dt.float32
BF16 = mybir.dt.bfloat16
FP8 = mybir.dt.float8e4
NP_BF16 = ml_dtypes.bfloat16

# fp8 scale plan for the out-matmul (phase E): e is stored as exp(z - ln4)
# (max ~166, inside e4m3 range) and vts as v * r * ALPHA_V (O(1) values).
# accum row sums are of exp(z - ln4), so r = 4/s; net output scale is
# 4 * ALPHA_V * (1/4) = ALPHA_V, undone at PSUM eviction.
ALPHA_V = 8192.0
E_BIAS = -1.3862943611198906  # -ln(4)


def build_bass():
    nc = bacc.Bacc(None, target_bir_lowering=False)

    xs_d = nc.dram_tensor("xs", [CH, N], BF16, kind="ExternalInput")
    xda_d = nc.dram_tensor("xda", [DEM + 1, NI], BF16, kind="ExternalInput")
    wq_d = nc.dram_tensor("wq", [CH, CH], BF16, kind="ExternalInput")
    wk_d = nc.dram_tensor("wk", [DEM + 1, CH], BF16, kind="ExternalInput")
    wv_d = nc.dram_tensor("wv", [DEM + 1, CH], BF16, kind="ExternalInput")
    out_d = nc.dram_tensor("out", [CH, N], F32, kind="ExternalOutput")

    xs_v = xs_d.ap().rearrange("(ko p) j -> p ko j", p=P)
    wq_v = wq_d.ap().rearrange("(ko p) m -> p ko m", p=P)
    out_v = out_d.ap().rearrange("(m p) j -> p m j", p=P)

    with tile.TileContext(nc) as tc:
        with (
            tc.tile_pool(name="consts", bufs=1) as consts,
            tc.tile_pool(name="bigs", bufs=1) as bigs,
            tc.tile_pool(name="small", bufs=1) as small,
            tc.tile_pool(name="stage", bufs=4) as stage,
        ):
            # Phase A needs only wk + xda: issue those DMAs first.
            wk_sb = consts.tile([DEM + 1, CH], BF16)
            nc.sync.dma_start(out=wk_sb, in_=wk_d.ap())
            xda_sb = consts.tile([DEM + 1, NI], BF16)
            nc.sync.dma_start(out=xda_sb, in_=xda_d.ap())
            wv_sb = consts.tile([DEM + 1, CH], BF16)
            nc.sync.dma_start(out=wv_sb, in_=wv_d.ap())
            wq_sb = consts.tile([P, KO, CH], BF16)
            nc.sync.dma_start(out=wq_sb, in_=wq_v)

            xs_sb = bigs.tile([P, KO, N], BF16)
            for ko in range(KO):
                nc.sync.dma_start(out=xs_sb[:, ko, :], in_=xs_v[:, ko, :])

            k_sb = bigs.tile([P, KO, NI], BF16)    # k[o, i], o on partitions
            kq_sb = bigs.tile([P, KO, NI], BF16)   # kq[ci, i], ci on partitions
            vt_sb = bigs.tile([P, NIB, CH], BF16)  # v^T[i, ch], i on partitions
            e_sb = bigs.tile([P, NIB, N], FP8)     # exp(logits - ln4)[i, j]
            vts_sb = bigs.tile([P, NIB, CH], FP8)  # v^T * r * ALPHA_V

            r_sb = small.tile([P, NIB], F32)
            sums_sb = small.tile([P, NIB, N // 2048], F32)
            ebias_sb = small.tile([P, 1], F32)
            nc.vector.memset(ebias_sb, E_BIAS)

            with tc.tile_pool(name="mm_psum", bufs=2, space="PSUM") as mm_psum:
                # ---- Phase A: k = [Wk^T; bk]^T @ [xd; 1] -> k_sb [o, i] ----
                for m in range(KO):
                    ps = mm_psum.tile([P, 2048], F32, tag="ps")
                    for i4 in range(NI // 512):
                        nc.tensor.matmul(
                            ps[:, i4 * 512:(i4 + 1) * 512],
                            lhsT=wk_sb[:, m * P:(m + 1) * P],
                            rhs=xda_sb[:, i4 * 512:(i4 + 1) * 512],
                            start=True, stop=True,
                        )
                    nc.vector.tensor_copy(out=k_sb[:, m, :], in_=ps)

                # ---- Phase B: v^T = [xd; 1]^T @ [Wv^T; bv] -> vt_sb ----
                for i4 in range(4):
                    ps = mm_psum.tile([P, 2048], F32, tag="ps")
                    for q in range(4):
                        ib = i4 * 4 + q
                        nc.tensor.matmul(
                            ps[:, q * 512:q * 512 + CH],
                            lhsT=xda_sb[:, ib * P:(ib + 1) * P],
                            rhs=wv_sb,
                            start=True, stop=True,
                        )
                        nc.vector.tensor_copy(
                            out=vt_sb[:, ib, :], in_=ps[:, q * 512:q * 512 + CH]
                        )

                # ---- Phase C: kq[ci, i] = sum_o (Wq*c)[o, ci] k[o, i] ----
                for m in range(KO):
                    ps = mm_psum.tile([P, 2048], F32, tag="ps")
                    for ko in range(KO):
                        for jj in range(4):
                            s0 = jj * 512
                            nc.tensor.matmul(
                                ps[:, s0:s0 + 512],
                                lhsT=wq_sb[:, ko, m * P:(m + 1) * P],
                                rhs=k_sb[:, ko, s0:s0 + 512],
                                start=(ko == 0), stop=(ko == 1),
                            )
                    nc.vector.tensor_copy(out=kq_sb[:, m, :], in_=ps)

                # ---- Phase D: logits -> exp -> row sums -> scale v^T ----
                for ib in range(NIB):
                    for jp in range(N // 2048):
                        pp = mm_psum.tile([P, 2048], F32, tag="ps")
                        for ko in range(KO):
                            for jj in range(4):
                                j0 = jp * 2048 + jj * 512
                                nc.tensor.matmul(
                                    pp[:, jj * 512:(jj + 1) * 512],
                                    lhsT=kq_sb[:, ko, ib * P:(ib + 1) * P],
                                    rhs=xs_sb[:, ko, j0:j0 + 512],
                                    start=(ko == 0), stop=(ko == 1),
                                )
                        nc.scalar.activation(
                            out=e_sb[:, ib, jp * 2048:(jp + 1) * 2048],
                            in_=pp,
                            func=mybir.ActivationFunctionType.Exp,
                            bias=ebias_sb,
                            accum_out=sums_sb[:, ib, jp:jp + 1],
                        )
                    nc.vector.reduce_sum(
                        out=r_sb[:, ib:ib + 1],
                        in_=sums_sb[:, ib, :],
                        axis=mybir.AxisListType.X,
                    )
                    nc.vector.reciprocal(
                        out=r_sb[:, ib:ib + 1], in_=r_sb[:, ib:ib + 1]
                    )
                    nc.vector.tensor_scalar(
                        out=vts_sb[:, ib, :],
                        in0=vt_sb[:, ib, :],
                        scalar1=r_sb[:, ib:ib + 1],
                        scalar2=ALPHA_V,
                        op0=mybir.AluOpType.mult,
                        op1=mybir.AluOpType.mult,
                    )

            # ---- Phase E: out[ch, j] = sum_i vts[i, ch] e[i, j] ----
            with tc.tile_pool(name="out_psum", bufs=4, space="PSUM") as out_psum:
                NPAIR = NIB // 2
                for jq in range(4):
                    pq = [out_psum.tile([P, 512], F32, tag="po",
                                        name=f"po_{jq}_{t}")
                          for t in range(4)]
                    for ibp in range(NPAIR):
                        for m in range(KO):
                            for jj in range(2):
                                jn = jq * 2 + jj
                                nc.tensor.matmul(
                                    pq[m * 2 + jj],
                                    lhsT=vts_sb[:, 2 * ibp:2 * ibp + 2,
                                                m * P:(m + 1) * P],
                                    rhs=e_sb[:, 2 * ibp:2 * ibp + 2,
                                             jn * 512:(jn + 1) * 512],
                                    start=(ibp == 0), stop=(ibp == NPAIR - 1),
                                    perf_mode=mybir.MatmulPerfMode.DoubleRow,
                                )
                    for m in range(KO):
                        for jj in range(2):
                            jn = jq * 2 + jj
                            st = stage.tile([P, 512], F32, tag="st")
                            nc.vector.tensor_scalar_mul(
                                out=st, in0=pq[m * 2 + jj],
                                scalar1=1.0 / ALPHA_V,
                            )
                            nc.sync.dma_start(
                                out=out_v[:, m, jn * 512:(jn + 1) * 512], in_=st
                            )
    nc.finalize()
    return nc


_NC_CACHE = None


def _get_nc():
    global _NC_CACHE
    if _NC_CACHE is None:
        _NC_CACHE = build_bass()
    return _NC_CACHE


def make_in_maps(x_s2, x_dem, Wq, bq, Wk, bk, Wv, bv):
    scale = np.float32(CH ** -0.5)
    wq = np.ascontiguousarray(Wq * scale).astype(NP_BF16)                # [o, ci]
    wk = np.concatenate([Wk.T, bk[None, :]], axis=0).astype(NP_BF16)     # [65, 256]
    wv = np.concatenate([Wv.T, bv[None, :]], axis=0).astype(NP_BF16)
    ones = np.ones((1, NI), np.float32)
    in_maps = []
    for c in range(NCORES):
        s, h = divmod(c, 2)
        xs = np.ascontiguousarray(x_s2[s].reshape(CH, N)).astype(NP_BF16)
        xd = x_dem[s].reshape(DEM, N)[:, h * NI:(h + 1) * NI]
        xda = np.concatenate([xd, ones], axis=0).astype(NP_BF16)
        in_maps.append({"xs": xs, "xda": np.ascontiguousarray(xda),
                        "wq": wq, "wk": wk, "wv": wv})
    return in_maps


def run(inputs, trace=False, trace_cores=None):
    """Run the device kernel; returns (output, BassKernelResults)."""
    x_s2 = np.asarray(inputs["x_s2"], np.float32)
    x_dem = np.asarray(inputs["x_dem"], np.float32)
    args = {k: np.asarray(inputs[k], np.float32)
            for k in ("Wq", "bq", "Wk", "bk", "Wv", "bv")}
    in_maps = make_in_maps(x_s2, x_dem, args["Wq"], args["bq"],
                           args["Wk"], args["bk"], args["Wv"], args["bv"])
    nc = _get_nc()
    res = run_bass_kernel_spmd(nc, in_maps, core_ids=list(range(NCORES)),
                               trace=trace, trace_cores=trace_cores)
    B = x_s2.shape[0]
    out = np.empty_like(x_s2)
    for s in range(B):
        part = res.results[2 * s]["out"] + res.results[2 * s + 1]["out"]
        out[s] = part.reshape(CH, 64, 64) + x_s2[s]
    return out, res


def kernel(**inputs):
    out, _ = run(inputs, trace=False)
    return out


# revision 32
# speedup vs baseline: 1.2473x; 1.0171x over previous
"""Cross-attention fusion kernel for Trainium2 (8 NeuronCores).

Reference computation (per sample b):
    q = Wq @ xs + bq            xs = x_s2[b] as [256, 4096]
    k = Wk @ xd + bk            xd = x_dem[b] as [64, 4096]
    v = Wv @ xd + bv
    attn = softmax_j(k^T q * c)             c = 256 ** -0.5
    out = v @ attn + x_s2[b]                out[ch, j] = sum_i v[ch, i] attn[i, j]

Device-side restructure (mathematically identical):
  - kq = (Wq * c)^T @ k, so logits = kq^T @ xs and q never materializes.
  - bq adds a per-i constant to logits, which cancels in softmax_j -> dropped.
  - bk / bv folded in by augmenting xd with a ones row (contraction K=65).
  - softmax denominators folded into v columns (scale v[:, i] by 1/sum_j e[i, j])
    instead of scaling the whole e matrix.
  - exp logits are left unshifted (no running-max): logits are O(1) here and
    fp32 exp has ~1e38 of headroom.

Sharding: 8 cores = 4 samples x 2 halves of the key-pixel axis i. Each core
emits a partial out [256, 4096]; the host sums the two halves and adds the
residual. No collectives.
"""

import numpy as np
import ml_dtypes

import concourse.bass as bass
import concourse.mybir as mybir
import concourse.tile as tile
from concourse import bacc
from concourse.bass_utils import run_bass_kernel_spmd

P = 128
CH = 256          # out_ch == s2_ch
DEM = 64          # dem_ch
N = 4096          # pixels per sample (j axis)
NI = 2048         # key pixels per core (i axis, half of N)
KO = CH // P      # 2 partition chunks of the 256-channel axis
NIB = NI // P     # 16 i-blocks per core
NCORES = 8

F32 = mybir.dt.float32
BF16 = mybir.dt.bfloat16
FP8 = mybir.dt.float8e4
NP_BF16 = ml_dtypes.bfloat16

# fp8 scale plan for the out-matmul (phase E): e is stored as exp(z - ln4)
# (max ~166, inside e4m3 range) and vts as v * r * ALPHA_V (O(1) values).
# accum row sums are of exp(z - ln4), so r = 4/s; net output scale is
# 4 * ALPHA_V * (1/4) = ALPHA_V, undone at PSUM eviction.
ALPHA_V = 8192.0
E_BIAS = -1.3862943611198906  # -ln(4)


def build_bass():
    nc = bacc.Bacc(None, target_bir_lowering=False)

    xs_d = nc.dram_tensor("xs", [CH, N], BF16, kind="ExternalInput")
    xda_d = nc.dram_tensor("xda", [DEM + 1, NI], BF16, kind="ExternalInput")
    wq_d = nc.dram_tensor("wq", [CH, CH], BF16, kind="ExternalInput")
    wk_d = nc.dram_tensor("wk", [DEM + 1, CH], BF16, kind="ExternalInput")
    wv_d = nc.dram_tensor("wv", [DEM + 1, CH], BF16, kind="ExternalInput")
    out_d = nc.dram_tensor("out", [CH, N], F32, kind="ExternalOutput")

    xs_v = xs_d.ap().rearrange("(ko p) j -> p ko j", p=P)
    wq_v = wq_d.ap().rearrange("(ko p) m -> p ko m", p=P)
    out_v = out_d.ap().rearrange("(m p) j -> p m j", p=P)

    with tile.TileContext(nc) as tc:
        with (
            tc.tile_pool(name="consts", bufs=1) as consts,
            tc.tile_pool(name="bigs", bufs=1) as bigs,
            tc.tile_pool(name="small", bufs=1) as small,
            tc.tile_pool(name="stage", bufs=4) as stage,
        ):
            # Phase A needs only wk + xda: issue those DMAs first (xda in
            # 512-column chunks so the first k-GEMM can start early).
            wk_sb = consts.tile([DEM + 1, CH], BF16)
            nc.sync.dma_start(out=wk_sb, in_=wk_d.ap())
            xda_sb = consts.tile([DEM + 1, NI], BF16)
            for i4 in range(NI // 512):
                nc.sync.dma_start(
                    out=xda_sb[:, i4 * 512:(i4 + 1) * 512],
                    in_=xda_d.ap()[:, i4 * 512:(i4 + 1) * 512],
                )
            wv_sb = consts.tile([DEM + 1, CH], BF16)
            nc.sync.dma_start(out=wv_sb, in_=wv_d.ap())
            wq_sb = consts.tile([P, KO, CH], BF16)
            nc.sync.dma_start(out=wq_sb, in_=wq_v)

            xs_sb = bigs.tile([P, KO, N], BF16)
            for ko in range(KO):
                nc.sync.dma_start(out=xs_sb[:, ko, :], in_=xs_v[:, ko, :])

            k_sb = bigs.tile([P, KO, NI], BF16)    # k[o, i], o on partitions
            kq_sb = bigs.tile([P, KO, NI], BF16)   # kq[ci, i], ci on partitions
            vt_sb = bigs.tile([P, NIB, CH], BF16)  # v^T[i, ch], i on partitions
            e_sb = bigs.tile([P, NIB, N], FP8)     # exp(logits - ln4)[i, j]
            vts_sb = bigs.tile([P, NIB, CH], FP8)  # v^T * r * ALPHA_V

            r_sb = small.tile([P, NIB], F32)
            sums_sb = small.tile([P, NIB, N // 2048], F32)
            ebias_sb = small.tile([P, 1], F32)
            nc.vector.memset(ebias_sb, E_BIAS)
            warm_sb = small.tile([P, 512], BF16)
            nc.vector.memset(warm_sb, 0.0)

            with tc.tile_pool(name="mm_psum", bufs=2, space="PSUM") as mm_psum:
                # Warm the PE's HAM clock gate with throwaway matmuls while
                # the input DMAs are in flight (~3.4us of PE activity flips
                # the clock from 1.2 to 2.4 GHz).
                wp = mm_psum.tile([P, 2048], F32, tag="ps")
                for w in range(8):
                    nc.tensor.matmul(
                        wp[:, (w % 4) * 512:(w % 4) * 512 + 512],
                        lhsT=warm_sb[:, :P],
                        rhs=warm_sb,
                        start=True, stop=True,
                    )

                # ---- Phase A: k = [Wk^T; bk]^T @ [xd; 1] -> k_sb [o, i] ----
                for m in range(KO):
                    ps = mm_psum.tile([P, 2048], F32, tag="ps")
                    for i4 in range(NI // 512):
                        nc.tensor.matmul(
                            ps[:, i4 * 512:(i4 + 1) * 512],
                            lhsT=wk_sb[:, m * P:(m + 1) * P],
                            rhs=xda_sb[:, i4 * 512:(i4 + 1) * 512],
                            start=True, stop=True,
                        )
                    nc.scalar.copy(out=k_sb[:, m, :], in_=ps)

                # ---- Phase B: v^T = [xd; 1]^T @ [Wv^T; bv] -> vt_sb ----
                for i4 in range(4):
                    ps = mm_psum.tile([P, 2048], F32, tag="ps")
                    for q in range(4):
                        ib = i4 * 4 + q
                        nc.tensor.matmul(
                            ps[:, q * 512:q * 512 + CH],
                            lhsT=xda_sb[:, ib * P:(ib + 1) * P],
                            rhs=wv_sb,
                            start=True, stop=True,
                        )
                        nc.vector.tensor_copy(
                            out=vt_sb[:, ib, :], in_=ps[:, q * 512:q * 512 + CH]
                        )

                # ---- Phase C: kq[ci, i] = sum_o (Wq*c)[o, ci] k[o, i] ----
                for m in range(KO):
                    ps = mm_psum.tile([P, 2048], F32, tag="ps")
                    for ko in range(KO):
                        for jj in range(4):
                            s0 = jj * 512
                            nc.tensor.matmul(
                                ps[:, s0:s0 + 512],
                                lhsT=wq_sb[:, ko, m * P:(m + 1) * P],
                                rhs=k_sb[:, ko, s0:s0 + 512],
                                start=(ko == 0), stop=(ko == 1),
                            )
                    nc.scalar.copy(out=kq_sb[:, m, :], in_=ps)

                # ---- Phase D: logits -> exp -> row sums -> scale v^T ----
                for ib in range(NIB):
                    for jp in range(N // 2048):
                        pp = mm_psum.tile([P, 2048], F32, tag="ps")
                        for ko in range(KO):
                            for jj in range(4):
                                j0 = jp * 2048 + jj * 512
                                nc.tensor.matmul(
                                    pp[:, jj * 512:(jj + 1) * 512],
                                    lhsT=kq_sb[:, ko, ib * P:(ib + 1) * P],
                                    rhs=xs_sb[:, ko, j0:j0 + 512],
                                    start=(ko == 0), stop=(ko == 1),
                                )
                        nc.scalar.activation(
                            out=e_sb[:, ib, jp * 2048:(jp + 1) * 2048],
                            in_=pp,
                            func=mybir.ActivationFunctionType.Exp,
                            bias=ebias_sb,
                            accum_out=sums_sb[:, ib, jp:jp + 1],
                        )
                    nc.vector.reduce_sum(
                        out=r_sb[:, ib:ib + 1],
                        in_=sums_sb[:, ib, :],
                        axis=mybir.AxisListType.X,
                    )
                    nc.vector.reciprocal(
                        out=r_sb[:, ib:ib + 1], in_=r_sb[:, ib:ib + 1]
                    )
                    nc.vector.tensor_scalar(
                        out=vts_sb[:, ib, :],
                        in0=vt_sb[:, ib, :],
                        scalar1=r_sb[:, ib:ib + 1],
                        scalar2=ALPHA_V,
                        op0=mybir.AluOpType.mult,
                        op1=mybir.AluOpType.mult,
                    )

            # ---- Phase E: out[ch, j] = sum_i vts[i, ch] e[i, j] ----
            with tc.tile_pool(name="out_psum", bufs=4, space="PSUM") as out_psum:
                NPAIR = NIB // 2
                for jq in range(4):
                    pq = [out_psum.tile([P, 512], F32, tag="po",
                                        name=f"po_{jq}_{t}")
                          for t in range(4)]
                    for ibp in range(NPAIR):
                        for m in range(KO):
                            for jj in range(2):
                                jn = jq * 2 + jj
                                nc.tensor.matmul(
                                    pq[m * 2 + jj],
                                    lhsT=vts_sb[:, 2 * ibp:2 * ibp + 2,
                                                m * P:(m + 1) * P],
                                    rhs=e_sb[:, 2 * ibp:2 * ibp + 2,
                                             jn * 512:(jn + 1) * 512],
                                    start=(ibp == 0), stop=(ibp == NPAIR - 1),
                                    perf_mode=mybir.MatmulPerfMode.DoubleRow,
                                )
                    for m in range(KO):
                        for jj in range(2):
                            jn = jq * 2 + jj
                            st = stage.tile([P, 512], F32, tag="st")
                            nc.vector.tensor_scalar_mul(
                                out=st, in0=pq[m * 2 + jj],
                                scalar1=1.0 / ALPHA_V,
                            )
                            nc.sync.dma_start(
                                out=out_v[:, m, jn * 512:(jn + 1) * 512], in_=st
                            )
    nc.finalize()
    return nc


_NC_CACHE = None


def _get_nc():
    global _NC_CACHE
    if _NC_CACHE is None:
        _NC_CACHE = build_bass()
    return _NC_CACHE


def make_in_maps(x_s2, x_dem, Wq, bq, Wk, bk, Wv, bv):
    scale = np.float32(CH ** -0.5)
    wq = np.ascontiguousarray(Wq * scale).astype(NP_BF16)                # [o, ci]
    wk = np.concatenate([Wk.T, bk[None, :]], axis=0).astype(NP_BF16)     # [65, 256]
    wv = np.concatenate([Wv.T, bv[None, :]], axis=0).astype(NP_BF16)
    ones = np.ones((1, NI), np.float32)
    in_maps = []
    for c in range(NCORES):
        s, h = divmod(c, 2)
        xs = np.ascontiguousarray(x_s2[s].reshape(CH, N)).astype(NP_BF16)
        xd = x_dem[s].reshape(DEM, N)[:, h * NI:(h + 1) * NI]
        xda = np.concatenate([xd, ones], axis=0).astype(NP_BF16)
        in_maps.append({"xs": xs, "xda": np.ascontiguousarray(xda),
                        "wq": wq, "wk": wk, "wv": wv})
    return in_maps


def run(inputs, trace=False, trace_cores=None):
    """Run the device kernel; returns (output, BassKernelResults)."""
    x_s2 = np.asarray(inputs["x_s2"], np.float32)
    x_dem = np.asarray(inputs["x_dem"], np.float32)
    args = {k: np.asarray(inputs[k], np.float32)
            for k in ("Wq", "bq", "Wk", "bk", "Wv", "bv")}
    in_maps = make_in_maps(x_s2, x_dem, args["Wq"], args["bq"],
                           args["Wk"], args["bk"], args["Wv"], args["bv"])
    nc = _get_nc()
    res = run_bass_kernel_spmd(nc, in_maps, core_ids=list(range(NCORES)),
                               trace=trace, trace_cores=trace_cores)
    B = x_s2.shape[0]
    out = np.empty_like(x_s2)
    for s in range(B):
        part = res.results[2 * s]["out"] + res.results[2 * s + 1]["out"]
        out[s] = part.reshape(CH, 64, 64) + x_s2[s]
    return out, res


def kernel(**inputs):
    out, _ = run(inputs, trace=False)
    return out


# revision 34
# speedup vs baseline: 1.2864x; 1.0313x over previous
"""Cross-attention fusion kernel for Trainium2 (8 NeuronCores).

Reference computation (per sample b):
    q = Wq @ xs + bq            xs = x_s2[b] as [256, 4096]
    k = Wk @ xd + bk            xd = x_dem[b] as [64, 4096]
    v = Wv @ xd + bv
    attn = softmax_j(k^T q * c)             c = 256 ** -0.5
    out = v @ attn + x_s2[b]                out[ch, j] = sum_i v[ch, i] attn[i, j]

Device-side restructure (mathematically identical):
  - logits = k^T q * c = (M^T xd_aug)^T xs with M = [Wk^T; bk] @ (Wq * c)
    precomputed on the host ([65, 256]); neither q nor k materializes.
  - bq adds a per-i constant to logits, which cancels in softmax_j -> dropped.
  - bk / bv folded in via a ones row appended to xd (contraction K=65).
  - softmax denominators folded into v columns (scale v[:, i] by 1/sum_j e)
    instead of scaling the whole e matrix.
  - exp is computed without a running-max shift: logits are O(1) here and the
    fp8 e-matrix is range-shifted by a fixed -ln(4) instead.
  - e and the scaled v^T are stored as fp8e4m3 and the out-matmul runs in
    DoubleRow mode (K=256 per instruction, 2 fp8 MACs/cell/cycle).

Sharding: 8 cores = 4 samples x 2 halves of the key-pixel axis i. Each core
emits a partial out [256, 4096]; the host sums the two halves and adds the
residual. No collectives.
"""

import numpy as np
import ml_dtypes

import concourse.bass as bass
import concourse.mybir as mybir
import concourse.tile as tile
from concourse import bacc
from concourse.bass_utils import run_bass_kernel_spmd

P = 128
CH = 256          # out_ch == s2_ch
DEM = 64          # dem_ch
N = 4096          # pixels per sample (j axis)
NI = 2048         # key pixels per core (i axis, half of N)
KO = CH // P      # 2 partition chunks of the 256-channel axis
NIB = NI // P     # 16 i-blocks per core
NPAIR = NIB // 2
NCORES = 8

F32 = mybir.dt.float32
BF16 = mybir.dt.bfloat16
FP8 = mybir.dt.float8e4
NP_BF16 = ml_dtypes.bfloat16

# fp8 scale plan for the out-matmul (phase E): e is stored as exp(z - ln4)
# (max ~166, inside e4m3 range) and vts as v * r * ALPHA_V (O(1) values).
# accum row sums are of exp(z - ln4), so r = 4/s; net output scale is
# 4 * ALPHA_V * (1/4) = ALPHA_V, undone at PSUM eviction.
ALPHA_V = 8192.0
E_BIAS = -1.3862943611198906  # -ln(4)


def build_bass():
    nc = bacc.Bacc(None, target_bir_lowering=False)

    xs_d = nc.dram_tensor("xs", [CH, N], BF16, kind="ExternalInput")
    xda_d = nc.dram_tensor("xda", [DEM + 1, NI], BF16, kind="ExternalInput")
    wm_d = nc.dram_tensor("wm", [DEM + 1, CH], BF16, kind="ExternalInput")
    wv_d = nc.dram_tensor("wv", [DEM + 1, CH], BF16, kind="ExternalInput")
    out_d = nc.dram_tensor("out", [CH, N], F32, kind="ExternalOutput")

    xs_v = xs_d.ap().rearrange("(ko p) j -> p ko j", p=P)
    out_v = out_d.ap().rearrange("(m p) j -> p m j", p=P)

    with tile.TileContext(nc) as tc:
        with (
            tc.tile_pool(name="consts", bufs=1) as consts,
            tc.tile_pool(name="bigs", bufs=1) as bigs,
            tc.tile_pool(name="small", bufs=1) as small,
            tc.tile_pool(name="stage", bufs=8) as stage,
        ):
            # Phase A needs only wm + xda: issue those DMAs first (xda in
            # 512-column chunks so the first GEMM can start early).
            wm_sb = consts.tile([DEM + 1, CH], BF16)
            nc.sync.dma_start(out=wm_sb, in_=wm_d.ap())
            xda_sb = consts.tile([DEM + 1, NI], BF16)
            for i4 in range(NI // 512):
                nc.sync.dma_start(
                    out=xda_sb[:, i4 * 512:(i4 + 1) * 512],
                    in_=xda_d.ap()[:, i4 * 512:(i4 + 1) * 512],
                )
            wv_sb = consts.tile([DEM + 1, CH], BF16)
            nc.sync.dma_start(out=wv_sb, in_=wv_d.ap())

            # xs ordered so phase D's first j-half arrives first
            xs_sb = bigs.tile([P, KO, N], BF16)
            for jh in range(2):
                for ko in range(KO):
                    nc.sync.dma_start(
                        out=xs_sb[:, ko, jh * 2048:(jh + 1) * 2048],
                        in_=xs_v[:, ko, jh * 2048:(jh + 1) * 2048],
                    )

            kq_sb = bigs.tile([P, KO, NI], BF16)   # kq[ci, i], ci on partitions
            vt_sb = bigs.tile([P, NIB, CH], BF16)  # v^T[i, ch], i on partitions
            e_sb = bigs.tile([P, NIB, N], FP8)     # exp(logits - ln4)[i, j]
            vts_sb = bigs.tile([P, NIB, CH], FP8)  # v^T * r * ALPHA_V

            r_sb = small.tile([P, NIB], F32)
            sums_sb = small.tile([P, NIB, N // 2048], F32)
            ebias_sb = small.tile([P, 1], F32)
            nc.vector.memset(ebias_sb, E_BIAS)
            warm_sb = small.tile([P, 512], BF16)
            nc.vector.memset(warm_sb, 0.0)

            with tc.tile_pool(name="mm_psum", bufs=2, space="PSUM") as mm_psum:
                # Warm the PE's HAM clock gate with throwaway matmuls while
                # the input DMAs are in flight (~3.4us of PE activity flips
                # the clock from 1.2 to 2.4 GHz).
                wp = mm_psum.tile([P, 2048], F32, tag="ps")
                for w in range(10):
                    nc.tensor.matmul(
                        wp[:, (w % 4) * 512:(w % 4) * 512 + 512],
                        lhsT=warm_sb[:, :P],
                        rhs=warm_sb,
                        start=True, stop=True,
                    )

                # ---- Phase A: kq[ci, i] = sum_r M[r, ci] xd_aug[r, i] ----
                for m in range(KO):
                    ps = mm_psum.tile([P, 2048], F32, tag="ps")
                    for i4 in range(NI // 512):
                        nc.tensor.matmul(
                            ps[:, i4 * 512:(i4 + 1) * 512],
                            lhsT=wm_sb[:, m * P:(m + 1) * P],
                            rhs=xda_sb[:, i4 * 512:(i4 + 1) * 512],
                            start=True, stop=True,
                        )
                    nc.scalar.copy(out=kq_sb[:, m, :], in_=ps)

                # ---- Phase B: v^T = [xd; 1]^T @ [Wv^T; bv] -> vt_sb ----
                for i4 in range(4):
                    ps = mm_psum.tile([P, 2048], F32, tag="ps")
                    for q in range(4):
                        ib = i4 * 4 + q
                        nc.tensor.matmul(
                            ps[:, q * 512:q * 512 + CH],
                            lhsT=xda_sb[:, ib * P:(ib + 1) * P],
                            rhs=wv_sb,
                            start=True, stop=True,
                        )
                        nc.vector.tensor_copy(
                            out=vt_sb[:, ib, :], in_=ps[:, q * 512:q * 512 + CH]
                        )

                # ---- Phase D: logits -> exp -> row sums -> scale v^T ----
                for ib in range(NIB):
                    for jp in range(N // 2048):
                        pp = mm_psum.tile([P, 2048], F32, tag="ps")
                        for ko in range(KO):
                            for jj in range(4):
                                j0 = jp * 2048 + jj * 512
                                nc.tensor.matmul(
                                    pp[:, jj * 512:(jj + 1) * 512],
                                    lhsT=kq_sb[:, ko, ib * P:(ib + 1) * P],
                                    rhs=xs_sb[:, ko, j0:j0 + 512],
                                    start=(ko == 0), stop=(ko == 1),
                                )
                        nc.scalar.activation(
                            out=e_sb[:, ib, jp * 2048:(jp + 1) * 2048],
                            in_=pp,
                            func=mybir.ActivationFunctionType.Exp,
                            bias=ebias_sb,
                            accum_out=sums_sb[:, ib, jp:jp + 1],
                        )
                    nc.vector.reduce_sum(
                        out=r_sb[:, ib:ib + 1],
                        in_=sums_sb[:, ib, :],
                        axis=mybir.AxisListType.X,
                    )
                    nc.vector.reciprocal(
                        out=r_sb[:, ib:ib + 1], in_=r_sb[:, ib:ib + 1]
                    )
                    nc.vector.tensor_scalar(
                        out=vts_sb[:, ib, :],
                        in0=vt_sb[:, ib, :],
                        scalar1=r_sb[:, ib:ib + 1],
                        scalar2=ALPHA_V,
                        op0=mybir.AluOpType.mult,
                        op1=mybir.AluOpType.mult,
                    )

            # ---- Phase E: out[ch, j] = sum_i vts[i, ch] e[i, j] ----
            # j-halves of 2048 with 8 live PSUM tiles, so each DoubleRow
            # weight load serves 4 matmuls.
            with tc.tile_pool(name="out_psum", bufs=8, space="PSUM") as out_psum:
                for jh in range(2):
                    pq = [out_psum.tile([P, 512], F32, tag="po",
                                        name=f"po_{jh}_{t}")
                          for t in range(8)]
                    for ibp in range(NPAIR):
                        for m in range(KO):
                            for jj in range(4):
                                jn = jh * 4 + jj
                                nc.tensor.matmul(
                                    pq[m * 4 + jj],
                                    lhsT=vts_sb[:, 2 * ibp:2 * ibp + 2,
                                                m * P:(m + 1) * P],
                                    rhs=e_sb[:, 2 * ibp:2 * ibp + 2,
                                             jn * 512:(jn + 1) * 512],
                                    start=(ibp == 0), stop=(ibp == NPAIR - 1),
                                    perf_mode=mybir.MatmulPerfMode.DoubleRow,
                                )
                    for m in range(KO):
                        for jj in range(4):
                            jn = jh * 4 + jj
                            st = stage.tile([P, 512], F32, tag="st",
                                            name=f"st_{jh}_{m}_{jj}")
                            nc.vector.tensor_scalar_mul(
                                out=st, in0=pq[m * 4 + jj],
                                scalar1=1.0 / ALPHA_V,
                            )
                            nc.sync.dma_start(
                                out=out_v[:, m, jn * 512:(jn + 1) * 512], in_=st
                            )
    nc.finalize()
    return nc


_NC_CACHE = None


def _get_nc():
    global _NC_CACHE
    if _NC_CACHE is None:
        _NC_CACHE = build_bass()
    return _NC_CACHE


def make_in_maps(x_s2, x_dem, Wq, bq, Wk, bk, Wv, bv):
    scale = np.float32(CH ** -0.5)
    wk_aug = np.concatenate([Wk.T, bk[None, :]], axis=0)                 # [65, 256]
    wm = (wk_aug @ (Wq * scale)).astype(NP_BF16)                         # [65, 256]
    wv_aug = np.concatenate([Wv.T, bv[None, :]], axis=0).astype(NP_BF16)
    ones = np.ones((1, NI), np.float32)
    in_maps = []
    for c in range(NCORES):
        s, h = divmod(c, 2)
        xs = np.ascontiguousarray(x_s2[s].reshape(CH, N)).astype(NP_BF16)
        xd = x_dem[s].reshape(DEM, N)[:, h * NI:(h + 1) * NI]
        xda = np.concatenate([xd, ones], axis=0).astype(NP_BF16)
        in_maps.append({"xs": xs, "xda": np.ascontiguousarray(xda),
                        "wm": wm, "wv": wv_aug})
    return in_maps


def run(inputs, trace=False, trace_cores=None):
    """Run the device kernel; returns (output, BassKernelResults)."""
    x_s2 = np.asarray(inputs["x_s2"], np.float32)
    x_dem = np.asarray(inputs["x_dem"], np.float32)
    args = {k: np.asarray(inputs[k], np.float32)
            for k in ("Wq", "bq", "Wk", "bk", "Wv", "bv")}
    in_maps = make_in_maps(x_s2, x_dem, args["Wq"], args["bq"],
                           args["Wk"], args["bk"], args["Wv"], args["bv"])
    nc = _get_nc()
    res = run_bass_kernel_spmd(nc, in_maps, core_ids=list(range(NCORES)),
                               trace=trace, trace_cores=trace_cores)
    B = x_s2.shape[0]
    out = np.empty_like(x_s2)
    for s in range(B):
        part = res.results[2 * s]["out"] + res.results[2 * s + 1]["out"]
        out[s] = part.reshape(CH, 64, 64) + x_s2[s]
    return out, res


def kernel(**inputs):
    out, _ = run(inputs, trace=False)
    return out
